# revision 8
# baseline (speedup 1.0000x reference)
"""Class-balanced segmentation loss on 8 Trainium2 NeuronCores.

Math: with counts_c = #{p: t_p == c}, S = sum_p lse_p, T_c = sum_{t_p=c}
pred[c, p], and w_c = 0.001 / (1 - 0.999**counts_c) (0 for empty classes):

    loss = (sum_c w_c * (S_c - T_c)) / (sum_c w_c * counts_c)

The histogram over the integer target is computed on the host (np.bincount
while laying out/sharding the inputs); the weights w_c then multiply the
per-class DEVICE partials, so the device never needs a counts pass:

  - fast path (all pixels valid, all w_c equal -- always true for this
    data regime, where 0.999**counts underflows and every w_c == 0.001):
    numerator = w * (sum_p lse_p - sum_c T_c), denominator = w * N.
    sum_p lse_p rides the ln activation's accum_out for free.
  - general path (ignore_index pixels or unequal weights): a per-pixel
    weight map W_p = w_(t_p) * valid_p is prepared on the host, and the
    device computes sum_p W_p*lse_p with one extra tensor_tensor per chunk.

Device pass per core (one batch; pixels on partitions, [128, 19, F] chunks):
  exp (one ACT instr over all 19 classes) -> sumexp over classes (DVE
  tensor_tensor chain, optional gpsimd side-chain) -> ln with accum_out
  (ACT) -> per class one fused STT (t==c)*pred_c with accum_out -> DMA out
  the [128, 19*NCH] fp32 T partials + [128, NCH] S partials.
"""

import os

import numpy as np

NCLASS = 19
B, H, W = 8, 512, 512
NPIX = H * W          # 262144 pixels per batch
P = 128               # SBUF partitions
FW = NPIX // P        # 2048 free-dim elements per partition
NCORES = 8
BETA = 1.0 - 0.001

F = int(os.environ.get("CHUNK_F", "1024"))  # free-dim chunk size
NCH = FW // F                               # chunks per batch
GPS_SUMEXP = int(os.environ.get("GPS_SUMEXP", "0"))  # adds on gpsimd
SUMEXP_TREE = int(os.environ.get("SUMEXP_TREE", "0"))
SKIP_STT = int(os.environ.get("SKIP_STT", "0"))      # attribution probes
SKIP_SUMEXP = int(os.environ.get("SKIP_SUMEXP", "0"))
SKIP_EXP = int(os.environ.get("SKIP_EXP", "0"))

_COMPILED = {}


def _np_bf16():
    import ml_dtypes

    return ml_dtypes.bfloat16


def _patch_tile_drain():
    """walrus in this container rejects >1 sem-wait on one instruction
    ("Too many sync wait commands"); the tile-exit Drain carries one wait
    per logical processor. Split them into single-wait NOPs."""
    import bass_rust
    import concourse.tile as tile

    if getattr(tile.TileContext, "_drain_patched", False):
        return

    def _drain_and_barrier(self, tick_clock, wait_clock):
        from concourse.tile import ScopedClock

        probe = self.nc.sync.nop(nofuse=True)
        wait_clock.add_sem_waits(
            probe.ins, ScopedClock({None: tick_clock.global_clock})
        )
        si = probe.ins.sync_info
        waits = list(si.on_wait) if si else []
        if si:
            si.on_wait = waits[:1]
        for i in range(1, len(waits)):
            n = self.nc.sync.nop(nofuse=True)
            n.ins.sync_info = bass_rust.SyncInfo(
                on_wait=waits[i : i + 1], on_update=[]
            )
        self.nc.sync.drain()
        self.nc.all_engine_barrier()
        assert self.sems is not None
        popped = self.nc._tile_sem_poison_stack.pop()
        assert popped is self._sem_poison
        self.nc.clear_and_free_semaphores(list(self.sems.allocated().values()))
        self.nc.all_engine_barrier()

    tile.TileContext._drain_and_barrier = _drain_and_barrier
    tile.TileContext._drain_patched = True


def _split_excess_waits(nc, maxw=1):
    """Post-pass: any instruction carrying more than `maxw` sem-waits gets
    the extras moved onto same-engine NOPs inserted right before it (the
    engine executes in order, so semantics are identical)."""
    import bass_rust

    for blk in nc.m.functions[0].blocks:
        insts = list(blk.instructions)
        out = []
        changed = False
        for inst in insts:
            si = inst.sync_info
            if si is not None and si.on_wait and len(si.on_wait) > maxw:
                waits = list(si.on_wait)
                si.on_wait = waits[:maxw]
                extra = waits[maxw:]
                eng = nc.engines[inst.engine]
                for i in range(0, len(extra), maxw):
                    n = eng.nop(nofuse=True)
                    cur = nc.cur_bb.bb
                    cur_insts = list(cur.instructions)
                    assert cur_insts[-1].name == n.ins.name
                    cur.instructions = cur_insts[:-1]
                    n.ins.sync_info = bass_rust.SyncInfo(
                        on_wait=extra[i : i + maxw], on_update=[]
                    )
                    out.append(n.ins)
                changed = True
            out.append(inst)
        if changed:
            blk.instructions = out


def build_nc(reps: int = 1, general: bool = False):
    """Per-core Bass program (SPMD over 8 cores, one batch each).

    Inputs: pred [P, NCH, NCLASS, F] bf16 (class-major chunks, host
    pre-transposed), targ [P, FW] bf16 (class ids as floats; invalid
    pixels remapped to -5 so no is_equal matches). General variant adds
    wmap [P, FW] bf16 (per-pixel class weight, 0 for invalid).

    Outputs: out_t [P, NCH*NCLASS] fp32 (T partials, col k*19+c),
    out_s [P, NCH] fp32 (sum of lse per chunk; general: W-weighted).
    """
    from contextlib import ExitStack

    import concourse.bass as bass
    import concourse.tile as tile
    from concourse import mybir

    _patch_tile_drain()

    io_dt = mybir.dt.bfloat16
    nc = bass.Bass()
    pred = nc.declare_dram_parameter(
        "pred", [P, NCH, NCLASS, F], io_dt, isOutput=False
    )
    targ = nc.declare_dram_parameter("targ", [P, FW], io_dt, isOutput=False)
    if general:
        wmap = nc.declare_dram_parameter(
            "wmap", [P, FW], io_dt, isOutput=False
        )
    out_t = nc.declare_dram_parameter(
        "out_t", [P, NCH * NCLASS], mybir.dt.float32, isOutput=True
    )
    out_s = nc.declare_dram_parameter(
        "out_s", [P, NCH], mybir.dt.float32, isOutput=True
    )

    with tile.TileContext(nc) as tc:
        with ExitStack() as ctx:
            io = ctx.enter_context(tc.tile_pool(name="io", bufs=2))
            work = ctx.enter_context(tc.tile_pool(name="work", bufs=2))
            pp = ctx.enter_context(tc.tile_pool(name="pp", bufs=3))
            acc = ctx.enter_context(tc.tile_pool(name="acc", bufs=1))

            t_acc = acc.tile([P, NCH * NCLASS], mybir.dt.float32)
            nc.vector.memset(t_acc[:, :], 0.0)
            s_acc = acc.tile([P, NCH], mybir.dt.float32)
            nc.vector.memset(s_acc[:, :], 0.0)

            def _body():
                # Software pipeline, explicit per-engine program order:
                #   ACT: exp(0), exp(1), ..., ln(0), ln(1), ...
                #   DVE: STT(0), STT(1), ..., sumexp(0), sumexp(1), ...
                # so exp(k+1) never waits behind ln(k), and the STTs (which
                # only need the DMA) fill the DVE while ACT runs exp.
                p_tiles, t_tiles, e_tiles, w_tiles = [], [], [], []
                for k in range(NCH):
                    p_tile = io.tile([P, NCLASS, F], io_dt, tag="p")
                    nc.sync.dma_start(out=p_tile[:, :, :], in_=pred[:, k, :, :])
                    t_tile = io.tile([P, F], io_dt, tag="t")
                    nc.sync.dma_start(
                        out=t_tile[:], in_=targ[:, k * F : (k + 1) * F]
                    )
                    if general:
                        w_tile = io.tile([P, F], io_dt, tag="w")
                        nc.sync.dma_start(
                            out=w_tile[:], in_=wmap[:, k * F : (k + 1) * F]
                        )
                        w_tiles.append(w_tile)
                    p_tiles.append(p_tile)
                    t_tiles.append(t_tile)

                    # exp of all classes in one ACT instruction
                    e_tile = work.tile([P, NCLASS, F], io_dt, tag="e")
                    nc.scalar.activation(
                        out=e_tile[:, :, :],
                        in_=p_tile[:, :, :],
                        func=mybir.ActivationFunctionType.Exp
                        if not SKIP_EXP
                        else mybir.ActivationFunctionType.Copy,
                    )
                    e_tiles.append(e_tile)

                    # T partials: one fused compare+mul+reduce per class,
                    # independent of the exp/lse chain (only needs the DMA)
                    for c in range(NCLASS if not SKIP_STT else 0):
                        prod = pp.tile([P, F], io_dt, tag="prod")
                        col = k * NCLASS + c
                        nc.vector.scalar_tensor_tensor(
                            out=prod[:],
                            in0=t_tile[:],
                            scalar=float(c),
                            in1=p_tile[:, c, :],
                            op0=mybir.AluOpType.is_equal,
                            op1=mybir.AluOpType.mult,
                            accum_out=t_acc[:, col : col + 1],
                        )

                # sumexp over classes, then ln (ACT) behind all exps
                sxs = []
                for k in range(NCH):
                    e_tile = e_tiles[k]
                    sx = work.tile([P, F], io_dt, tag="sx")
                    if SKIP_SUMEXP:
                        nc.vector.tensor_tensor(
                            sx[:], e_tile[:, 0, :], e_tile[:, 1, :],
                            mybir.AluOpType.add,
                        )
                    else:
                        nc.vector.tensor_tensor(
                            sx[:], e_tile[:, 0, :], e_tile[:, 1, :],
                            mybir.AluOpType.add,
                        )
                        for c in range(2, NCLASS):
                            nc.vector.tensor_tensor(
                                sx[:], sx[:], e_tile[:, c, :],
                                mybir.AluOpType.add,
                            )
                    sxs.append(sx)

                for k in range(NCH):
                    lse = work.tile([P, F], io_dt, tag="lse")
                    if general:
                        nc.scalar.activation(
                            out=lse[:],
                            in_=sxs[k][:],
                            func=mybir.ActivationFunctionType.Ln,
                        )
                        wl = pp.tile([P, F], io_dt, tag="wl")
                        nc.vector.tensor_tensor_reduce(
                            out=wl[:],
                            in0=w_tiles[k][:],
                            in1=lse[:],
                            scale=1.0,
                            scalar=0.0,
                            op0=mybir.AluOpType.mult,
                            op1=mybir.AluOpType.add,
                            accum_out=s_acc[:, k : k + 1],
                        )
                    else:
                        # sum_f lse rides the ln's accum_out for free
                        nc.scalar.activation(
                            out=lse[:],
                            in_=sxs[k][:],
                            func=mybir.ActivationFunctionType.Ln,
                            accum_out=s_acc[:, k : k + 1],
                        )

            if reps == 1:
                _body()
            else:
                with tc.For_i(0, reps, 1):
                    _body()

            nc.sync.dma_start(out=out_t[:, :], in_=t_acc[:, :])
            nc.sync.dma_start(out=out_s[:, :], in_=s_acc[:, :])

    _split_excess_waits(nc, maxw=1)
    return nc


def get_nc(reps: int = 1, general: bool = False):
    key = (reps, general)
    if key not in _COMPILED:
        _COMPILED[key] = build_nc(reps, general)
    return _COMPILED[key]


def _class_weights(targ_np):
    """Histogram of the full target -> quantized class weights (bf16, so
    the device W map and the host denominator use identical values)."""
    valid = targ_np >= 0
    counts = np.bincount(
        targ_np[valid].ravel().astype(np.int64), minlength=NCLASS
    )[:NCLASS].astype(np.float64)
    with np.errstate(divide="ignore", over="ignore", under="ignore"):
        w = np.float32(1.0 - BETA) / (
            1.0 - np.float32(BETA) ** counts.astype(np.float32)
        )
    w = np.where(counts > 0, w, 0.0).astype(np.float32)
    wq = w.astype(_np_bf16()).astype(np.float64)
    return counts, wq, valid


def _shard_inputs(pred_np, targ_np, wmap=None, t_enc=None):
    dt = _np_bf16()
    if t_enc is None:
        t_enc = targ_np.astype(np.float32)
    in_maps = []
    for b in range(NCORES):
        # [19, 262144] -> [P, NCH, NCLASS, F]
        pb = pred_np[b].reshape(NCLASS, P, NCH, F).transpose(1, 2, 0, 3)
        m = {
            "pred": np.ascontiguousarray(pb).astype(dt),
            "targ": t_enc[b].reshape(P, FW).astype(dt),
        }
        if wmap is not None:
            m["wmap"] = wmap[b].reshape(P, FW).astype(dt)
        in_maps.append(m)
    return in_maps


def _run_device(pred_np, targ_np, reps: int = 1, in_maps=None, general=False):
    from concourse.bass_utils import run_bass_kernel_spmd

    nc = get_nc(reps, general)
    if in_maps is None:
        in_maps = _shard_inputs(pred_np, targ_np)
    res = run_bass_kernel_spmd(nc, in_maps, core_ids=list(range(NCORES)))
    return [
        (res.results[i]["out_t"], res.results[i]["out_s"])
        for i in range(NCORES)
    ]


def kernel(pred: np.ndarray, target: np.ndarray) -> np.ndarray:
    pred_np = np.asarray(pred, dtype=np.float32)
    targ_np = np.asarray(target)

    counts, wq, valid = _class_weights(targ_np)
    uniform = bool(valid.all()) and bool(
        np.all(wq == wq[0]) and wq[0] > 0
    )

    if uniform:
        in_maps = _shard_inputs(pred_np, targ_np)
        outs = _run_device(pred_np, targ_np, in_maps=in_maps, general=False)
        S = np.float64(0.0)
        T = np.zeros(NCLASS, np.float64)
        for ot, os_ in outs:
            T += (
                np.asarray(ot, np.float64)
                .reshape(P, NCH, NCLASS)
                .sum((0, 1))
            )
            S += np.asarray(os_, np.float64).sum()
        num = wq[0] * (S - float(T.sum()))
        den = wq[0] * float(counts.sum())
        return np.array(np.float32(num / den))

    # general path: per-pixel weight map; invalid pixels get weight 0 and
    # a target code (-5) that matches no class
    wmap = (wq[np.clip(targ_np, 0, NCLASS - 1)] * valid).astype(np.float32)
    t_enc = np.where(valid, targ_np, -5).astype(np.float32)
    in_maps = _shard_inputs(pred_np, targ_np, wmap=wmap, t_enc=t_enc)
    outs = _run_device(pred_np, targ_np, in_maps=in_maps, general=True)
    SW = np.float64(0.0)
    T = np.zeros(NCLASS, np.float64)
    for ot, os_ in outs:
        T += np.asarray(ot, np.float64).reshape(P, NCH, NCLASS).sum((0, 1))
        SW += np.asarray(os_, np.float64).sum()
    num = SW - float((wq * T).sum())
    den = float((wq * counts).sum())
    return np.array(np.float32(num / den))


# revision 9
# speedup vs baseline: 1.4183x; 1.4183x over previous
"""Class-balanced segmentation loss on 8 Trainium2 NeuronCores.

Math: with counts_c = #{p: t_p == c}, S = sum_p lse_p, T_c = sum_{t_p=c}
pred[c, p], and w_c = 0.001 / (1 - 0.999**counts_c) (0 for empty classes):

    loss = (sum_c w_c * (S_c - T_c)) / (sum_c w_c * counts_c)

The histogram over the integer target is computed on the host (np.bincount
while laying out/sharding the inputs); the weights w_c then multiply the
per-class DEVICE partials, so the device never needs a counts pass:

  - fast path (all pixels valid, all w_c equal -- always true for this
    data regime, where 0.999**counts underflows and every w_c == 0.001):
    numerator = w * (sum_p lse_p - sum_c T_c), denominator = w * N.
    sum_p lse_p rides the ln activation's accum_out for free.
  - general path (ignore_index pixels or unequal weights): a per-pixel
    weight map W_p = w_(t_p) * valid_p is prepared on the host, and the
    device computes sum_p W_p*lse_p with one extra tensor_tensor per chunk.

Device pass per core (one batch; pixels on partitions, [128, 19, F] chunks):
  exp (one ACT instr over all 19 classes) -> sumexp over classes (DVE
  tensor_tensor chain, optional gpsimd side-chain) -> ln with accum_out
  (ACT) -> per class one fused STT (t==c)*pred_c with accum_out -> DMA out
  the [128, 19*NCH] fp32 T partials + [128, NCH] S partials.
"""

import os

import numpy as np

NCLASS = 19
B, H, W = 8, 512, 512
NPIX = H * W          # 262144 pixels per batch
P = 128               # SBUF partitions
FW = NPIX // P        # 2048 free-dim elements per partition
NCORES = 8
BETA = 1.0 - 0.001

F = int(os.environ.get("CHUNK_F", "1024"))  # free-dim chunk size
NCH = FW // F                               # chunks per batch
GPS_SUMEXP = int(os.environ.get("GPS_SUMEXP", "0"))  # adds on gpsimd
SUMEXP_TREE = int(os.environ.get("SUMEXP_TREE", "0"))
SKIP_STT = int(os.environ.get("SKIP_STT", "0"))      # attribution probes
SKIP_SUMEXP = int(os.environ.get("SKIP_SUMEXP", "0"))
SKIP_EXP = int(os.environ.get("SKIP_EXP", "0"))

_COMPILED = {}


def _np_bf16():
    import ml_dtypes

    return ml_dtypes.bfloat16


def _patch_tile_drain():
    """walrus in this container rejects >1 sem-wait on one instruction
    ("Too many sync wait commands"); the tile-exit Drain carries one wait
    per logical processor. Split them into single-wait NOPs."""
    import bass_rust
    import concourse.tile as tile

    if getattr(tile.TileContext, "_drain_patched", False):
        return

    def _drain_and_barrier(self, tick_clock, wait_clock):
        from concourse.tile import ScopedClock

        probe = self.nc.sync.nop(nofuse=True)
        wait_clock.add_sem_waits(
            probe.ins, ScopedClock({None: tick_clock.global_clock})
        )
        si = probe.ins.sync_info
        waits = list(si.on_wait) if si else []
        if si:
            si.on_wait = waits[:1]
        for i in range(1, len(waits)):
            n = self.nc.sync.nop(nofuse=True)
            n.ins.sync_info = bass_rust.SyncInfo(
                on_wait=waits[i : i + 1], on_update=[]
            )
        self.nc.sync.drain()
        self.nc.all_engine_barrier()
        assert self.sems is not None
        popped = self.nc._tile_sem_poison_stack.pop()
        assert popped is self._sem_poison
        self.nc.clear_and_free_semaphores(list(self.sems.allocated().values()))
        self.nc.all_engine_barrier()

    tile.TileContext._drain_and_barrier = _drain_and_barrier
    tile.TileContext._drain_patched = True


def _split_excess_waits(nc, maxw=1):
    """Post-pass: any instruction carrying more than `maxw` sem-waits gets
    the extras moved onto same-engine NOPs inserted right before it (the
    engine executes in order, so semantics are identical)."""
    import bass_rust

    for blk in nc.m.functions[0].blocks:
        insts = list(blk.instructions)
        out = []
        changed = False
        for inst in insts:
            si = inst.sync_info
            if si is not None and si.on_wait and len(si.on_wait) > maxw:
                waits = list(si.on_wait)
                si.on_wait = waits[:maxw]
                extra = waits[maxw:]
                eng = nc.engines[inst.engine]
                for i in range(0, len(extra), maxw):
                    n = eng.nop(nofuse=True)
                    cur = nc.cur_bb.bb
                    cur_insts = list(cur.instructions)
                    assert cur_insts[-1].name == n.ins.name
                    cur.instructions = cur_insts[:-1]
                    n.ins.sync_info = bass_rust.SyncInfo(
                        on_wait=extra[i : i + maxw], on_update=[]
                    )
                    out.append(n.ins)
                changed = True
            out.append(inst)
        if changed:
            blk.instructions = out


def build_nc(reps: int = 1, general: bool = False):
    """Per-core Bass program (SPMD over 8 cores, one batch each).

    Inputs: pred [P, NCH, NCLASS, F] bf16 (class-major chunks, host
    pre-transposed), targ [P, FW] bf16 (class ids as floats; invalid
    pixels remapped to -5 so no is_equal matches). General variant adds
    wmap [P, FW] bf16 (per-pixel class weight, 0 for invalid).

    Outputs: out_t [P, NCH*NCLASS] fp32 (T partials, col k*19+c),
    out_s [P, NCH] fp32 (sum of lse per chunk; general: W-weighted).
    """
    from contextlib import ExitStack

    import concourse.bass as bass
    import concourse.tile as tile
    from concourse import mybir

    _patch_tile_drain()

    io_dt = mybir.dt.bfloat16
    nc = bass.Bass()
    pred = nc.declare_dram_parameter(
        "pred", [P, NCH, NCLASS, F], io_dt, isOutput=False
    )
    targ = nc.declare_dram_parameter("targ", [P, FW], io_dt, isOutput=False)
    if general:
        wmap = nc.declare_dram_parameter(
            "wmap", [P, FW], io_dt, isOutput=False
        )
    out_t = nc.declare_dram_parameter(
        "out_t", [P, NCH * NCLASS], mybir.dt.float32, isOutput=True
    )
    out_s = nc.declare_dram_parameter(
        "out_s", [P, NCH], mybir.dt.float32, isOutput=True
    )

    with tile.TileContext(nc) as tc:
        with ExitStack() as ctx:
            io = ctx.enter_context(tc.tile_pool(name="io", bufs=2))
            work = ctx.enter_context(tc.tile_pool(name="work", bufs=2))
            pp = ctx.enter_context(tc.tile_pool(name="pp", bufs=3))
            acc = ctx.enter_context(tc.tile_pool(name="acc", bufs=1))

            t_acc = acc.tile([P, NCH * NCLASS], mybir.dt.float32)
            nc.vector.memset(t_acc[:, :], 0.0)
            s_acc = acc.tile([P, NCH], mybir.dt.float32)
            nc.vector.memset(s_acc[:, :], 0.0)

            def _body():
                # Software pipeline, explicit per-engine program order:
                #   ACT: exp(0), exp(1), ..., ln(0), ln(1), ...
                #   DVE: STT(0), STT(1), ..., sumexp(0), sumexp(1), ...
                # so exp(k+1) never waits behind ln(k), and the STTs (which
                # only need the DMA) fill the DVE while ACT runs exp.
                p_tiles, t_tiles, e_tiles, w_tiles = [], [], [], []
                for k in range(NCH):
                    p_tile = io.tile([P, NCLASS, F], io_dt, tag="p")
                    nc.sync.dma_start(out=p_tile[:, :, :], in_=pred[:, k, :, :])
                    t_tile = io.tile([P, F], io_dt, tag="t")
                    nc.sync.dma_start(
                        out=t_tile[:], in_=targ[:, k * F : (k + 1) * F]
                    )
                    if general:
                        w_tile = io.tile([P, F], io_dt, tag="w")
                        nc.sync.dma_start(
                            out=w_tile[:], in_=wmap[:, k * F : (k + 1) * F]
                        )
                        w_tiles.append(w_tile)
                    p_tiles.append(p_tile)
                    t_tiles.append(t_tile)

                    # exp of all classes in one ACT instruction
                    if SKIP_EXP:
                        e_tiles.append(p_tile)  # probe: no ACT exp at all
                    else:
                        e_tile = work.tile([P, NCLASS, F], io_dt, tag="e")
                        nc.scalar.activation(
                            out=e_tile[:, :, :],
                            in_=p_tile[:, :, :],
                            func=mybir.ActivationFunctionType.Exp,
                        )
                        e_tiles.append(e_tile)

                    # T partials: one fused compare+mul+reduce per class,
                    # independent of the exp/lse chain (only needs the DMA)
                    for c in range(NCLASS if not SKIP_STT else 0):
                        prod = pp.tile([P, F], io_dt, tag="prod")
                        col = k * NCLASS + c
                        nc.vector.scalar_tensor_tensor(
                            out=prod[:],
                            in0=t_tile[:],
                            scalar=float(c),
                            in1=p_tile[:, c, :],
                            op0=mybir.AluOpType.is_equal,
                            op1=mybir.AluOpType.mult,
                            accum_out=t_acc[:, col : col + 1],
                        )

                # sumexp over classes, then ln (ACT) behind all exps
                sxs = []
                for k in range(NCH):
                    e_tile = e_tiles[k]
                    sx = work.tile([P, F], io_dt, tag="sx")
                    if SKIP_SUMEXP:
                        nc.vector.tensor_tensor(
                            sx[:], e_tile[:, 0, :], e_tile[:, 1, :],
                            mybir.AluOpType.add,
                        )
                    else:
                        nc.vector.tensor_tensor(
                            sx[:], e_tile[:, 0, :], e_tile[:, 1, :],
                            mybir.AluOpType.add,
                        )
                        for c in range(2, NCLASS):
                            nc.vector.tensor_tensor(
                                sx[:], sx[:], e_tile[:, c, :],
                                mybir.AluOpType.add,
                            )
                    sxs.append(sx)

                for k in range(NCH):
                    lse = work.tile([P, F], io_dt, tag="lse")
                    if general:
                        nc.scalar.activation(
                            out=lse[:],
                            in_=sxs[k][:],
                            func=mybir.ActivationFunctionType.Ln,
                        )
                        wl = pp.tile([P, F], io_dt, tag="wl")
                        nc.vector.tensor_tensor_reduce(
                            out=wl[:],
                            in0=w_tiles[k][:],
                            in1=lse[:],
                            scale=1.0,
                            scalar=0.0,
                            op0=mybir.AluOpType.mult,
                            op1=mybir.AluOpType.add,
                            accum_out=s_acc[:, k : k + 1],
                        )
                    else:
                        # sum_f lse rides the ln's accum_out for free
                        nc.scalar.activation(
                            out=lse[:],
                            in_=sxs[k][:],
                            func=mybir.ActivationFunctionType.Ln,
                            accum_out=s_acc[:, k : k + 1],
                        )

            if reps == 1:
                _body()
            else:
                with tc.For_i(0, reps, 1):
                    _body()

            nc.sync.dma_start(out=out_t[:, :], in_=t_acc[:, :])
            nc.sync.dma_start(out=out_s[:, :], in_=s_acc[:, :])

    _split_excess_waits(nc, maxw=1)
    return nc


def get_nc(reps: int = 1, general: bool = False):
    key = (reps, general)
    if key not in _COMPILED:
        _COMPILED[key] = build_nc(reps, general)
    return _COMPILED[key]


def _class_weights(targ_np):
    """Histogram of the full target -> quantized class weights (bf16, so
    the device W map and the host denominator use identical values)."""
    valid = targ_np >= 0
    counts = np.bincount(
        targ_np[valid].ravel().astype(np.int64), minlength=NCLASS
    )[:NCLASS].astype(np.float64)
    with np.errstate(divide="ignore", over="ignore", under="ignore"):
        w = np.float32(1.0 - BETA) / (
            1.0 - np.float32(BETA) ** counts.astype(np.float32)
        )
    w = np.where(counts > 0, w, 0.0).astype(np.float32)
    wq = w.astype(_np_bf16()).astype(np.float64)
    return counts, wq, valid


def _shard_inputs(pred_np, targ_np, wmap=None, t_enc=None):
    dt = _np_bf16()
    if t_enc is None:
        t_enc = targ_np.astype(np.float32)
    in_maps = []
    for b in range(NCORES):
        # [19, 262144] -> [P, NCH, NCLASS, F]
        pb = pred_np[b].reshape(NCLASS, P, NCH, F).transpose(1, 2, 0, 3)
        m = {
            "pred": np.ascontiguousarray(pb).astype(dt),
            "targ": t_enc[b].reshape(P, FW).astype(dt),
        }
        if wmap is not None:
            m["wmap"] = wmap[b].reshape(P, FW).astype(dt)
        in_maps.append(m)
    return in_maps


def _run_device(pred_np, targ_np, reps: int = 1, in_maps=None, general=False):
    from concourse.bass_utils import run_bass_kernel_spmd

    nc = get_nc(reps, general)
    if in_maps is None:
        in_maps = _shard_inputs(pred_np, targ_np)
    res = run_bass_kernel_spmd(nc, in_maps, core_ids=list(range(NCORES)))
    return [
        (res.results[i]["out_t"], res.results[i]["out_s"])
        for i in range(NCORES)
    ]


def kernel(pred: np.ndarray, target: np.ndarray) -> np.ndarray:
    pred_np = np.asarray(pred, dtype=np.float32)
    targ_np = np.asarray(target)

    counts, wq, valid = _class_weights(targ_np)
    uniform = bool(valid.all()) and bool(
        np.all(wq == wq[0]) and wq[0] > 0
    )

    if uniform:
        in_maps = _shard_inputs(pred_np, targ_np)
        outs = _run_device(pred_np, targ_np, in_maps=in_maps, general=False)
        S = np.float64(0.0)
        T = np.zeros(NCLASS, np.float64)
        for ot, os_ in outs:
            T += (
                np.asarray(ot, np.float64)
                .reshape(P, NCH, NCLASS)
                .sum((0, 1))
            )
            S += np.asarray(os_, np.float64).sum()
        num = wq[0] * (S - float(T.sum()))
        den = wq[0] * float(counts.sum())
        return np.array(np.float32(num / den))

    # general path: per-pixel weight map; invalid pixels get weight 0 and
    # a target code (-5) that matches no class
    wmap = (wq[np.clip(targ_np, 0, NCLASS - 1)] * valid).astype(np.float32)
    t_enc = np.where(valid, targ_np, -5).astype(np.float32)
    in_maps = _shard_inputs(pred_np, targ_np, wmap=wmap, t_enc=t_enc)
    outs = _run_device(pred_np, targ_np, in_maps=in_maps, general=True)
    SW = np.float64(0.0)
    T = np.zeros(NCLASS, np.float64)
    for ot, os_ in outs:
        T += np.asarray(ot, np.float64).reshape(P, NCH, NCLASS).sum((0, 1))
        SW += np.asarray(os_, np.float64).sum()
    num = SW - float((wq * T).sum())
    den = float((wq * counts).sum())
    return np.array(np.float32(num / den))
